# revision 2
# baseline (speedup 1.0000x reference)
"""ESM2 contact predictor head on 8 Trainium2 NeuronCores.

Computes out[b, i, j] = sigmoid(x[b,i] @ W @ x[b,j] + bias) for
x: (8, 2050, 320) f32, W: (320, 320) f32, bias: (1,) f32.

Sharding: data-parallel over batch — core c handles batch element c.

Per-core algorithm (all matmuls in float32r at full PE rate):
  host:  xt = zero-pad(x[c].T)  -> (384, 2050), D padded 320->384 = 3*128
         wp = zero-pad(W)       -> (384, 384)
  chip:  u = wp.T @ xt          -> (384, 2050)   == (x[c] @ W).T   (padded)
         for each 128-row strip i of the (2050, 2050) output:
             logits[i, j] = sum_k u[k, i] * xt[k, j]   (PE, PSUM f32 accum)
             out_strip = sigmoid(logits + bias)        (ScalarE from PSUM)
             DMA strip -> HBM
"""

import numpy as np

import concourse.bass as bass
import concourse.mybir as mybir
import concourse.tile as tile
from concourse import bacc
from concourse.bass_utils import run_bass_kernel_spmd

N_CORES = 8
B, L, D = 8, 2050, 320
DP = 384          # padded D (3 * 128)
KT = DP // 128    # 3 K-tiles
F32 = mybir.dt.float32
F32R = mybir.dt.float32r
SIG = mybir.ActivationFunctionType.Sigmoid

# j-dim tiling of the 2050 output columns: 2 pairs of (512+512) + tail 2
NJ_PAIRS = 2
J_TAIL = 2048

# i-dim strips: 16 full 128-row strips + one 2-row strip
I_STRIPS = [(s * 128, 128) for s in range(16)] + [(2048, 2)]

_cache = {}


def _build(bias_val: float):
    nc = bacc.Bacc("TRN2", target_bir_lowering=False, debug=False,
                   num_devices=N_CORES)
    xt_d = nc.dram_tensor("xt", [DP, L], F32R, kind="ExternalInput")
    w_d = nc.dram_tensor("w", [DP, DP], F32R, kind="ExternalInput")
    out_d = nc.dram_tensor("out", [L, L], F32, kind="ExternalOutput")

    xt_r = xt_d.ap().rearrange("(k p) n -> p k n", p=128)   # (128, 3, 2050)
    w_r = w_d.ap().rearrange("(k p) e -> p k e", p=128)     # (128, 3, 384)

    with tile.TileContext(nc) as tc:
        with (
            tc.tile_pool(name="persist", bufs=1) as pp,
            tc.tile_pool(name="outp", bufs=3) as outp,
            tc.tile_pool(name="psum", bufs=2, space="PSUM") as psp,
        ):
            bias_t = pp.tile([128, 1], F32)
            nc.vector.memset(bias_t[:], bias_val)

            w_sb = pp.tile([128, KT, DP], F32R)
            nc.sync.dma_start(w_sb[:], w_r)

            # x^T, chunked over columns so matmul1 can start early
            xt_sb = pp.tile([128, KT, L], F32R)
            for c0 in range(0, L, 512):
                c1 = min(c0 + 512, L)
                nc.sync.dma_start(xt_sb[:, :, c0:c1], xt_r[:, :, c0:c1])

            u_sb = pp.tile([128, KT, L], F32R)

            # ---- phase 1: u = wp.T @ xt  (u[e, i], e on partitions) ----
            for n0 in range(0, L, 512):
                n1 = min(n0 + 512, L)
                nsz = n1 - n0
                for et in range(KT):
                    ps1 = psp.tile([128, 512], F32, tag="small", bufs=2)
                    for k in range(KT):
                        nc.tensor.matmul(
                            ps1[:, :nsz],
                            lhsT=w_sb[:, k, et * 128:(et + 1) * 128],
                            rhs=xt_sb[:, k, n0:n1],
                            start=(k == 0), stop=(k == KT - 1),
                        )
                    nc.vector.tensor_copy(u_sb[:, et, n0:n1], ps1[:, :nsz])

            # ---- phase 2: logits strips -> sigmoid -> DMA out ----
            for (i0, isz) in I_STRIPS:
                strip = outp.tile([128, L], F32, tag="strip", bufs=3)
                for jp in range(NJ_PAIRS):
                    ps = psp.tile([128, 1024], F32, tag="pair", bufs=3)
                    for h in range(2):
                        j0 = jp * 1024 + h * 512
                        for k in range(KT):
                            nc.tensor.matmul(
                                ps[:isz, h * 512:(h + 1) * 512],
                                lhsT=u_sb[:, k, i0:i0 + isz],
                                rhs=xt_sb[:, k, j0:j0 + 512],
                                start=(k == 0), stop=(k == KT - 1),
                            )
                    nc.scalar.activation(
                        strip[:isz, jp * 1024:(jp + 1) * 1024],
                        ps[:isz, :], SIG, bias=bias_t[:isz, :],
                    )
                # tail columns 2048:2050
                pst = psp.tile([128, 512], F32, tag="small", bufs=2)
                for k in range(KT):
                    nc.tensor.matmul(
                        pst[:isz, :2],
                        lhsT=u_sb[:, k, i0:i0 + isz],
                        rhs=xt_sb[:, k, J_TAIL:L],
                        start=(k == 0), stop=(k == KT - 1),
                    )
                nc.scalar.activation(
                    strip[:isz, J_TAIL:L], pst[:isz, :2], SIG,
                    bias=bias_t[:isz, :],
                )
                nc.sync.dma_start(out_d.ap()[i0:i0 + isz, :], strip[:isz, :])

    nc.compile()
    return nc


last_results = None


def kernel(x, W, b, _trace=False):
    global last_results
    x = np.ascontiguousarray(np.asarray(x, dtype=np.float32))
    W = np.asarray(W, dtype=np.float32)
    b = np.asarray(b, dtype=np.float32)
    bias_val = float(b[0])

    key = bias_val
    if key not in _cache:
        _cache.clear()
        _cache[key] = _build(bias_val)
    nc = _cache[key]

    xt_all = np.zeros((B, DP, L), dtype=np.float32)
    xt_all[:, :D, :] = x.transpose(0, 2, 1)
    wp = np.zeros((DP, DP), dtype=np.float32)
    wp[:D, :D] = W

    in_maps = [{"xt": xt_all[c], "w": wp} for c in range(N_CORES)]
    res = run_bass_kernel_spmd(nc, in_maps, core_ids=list(range(N_CORES)),
                               trace=_trace)
    last_results = res
    out = np.stack([res.results[c]["out"] for c in range(N_CORES)], axis=0)
    return out.astype(np.float32, copy=False)
